# revision 18
# baseline (speedup 1.0000x reference)
"""Bass/Trainium2 kernel for nn_Attention3D (dense transformer attention over
a 16^3 volume, 8 heads, dim_head 64).

Strategy: head-parallel across the 8 NeuronCores (1 head per core).
Per core (head h):
    B_h   = scale * wq_h^T @ wk_h            (128x128, host-precomputed)
    g     = B_h^T @ x                        (128, 4096)   on device
    S^T   chunks = x_chunk^T @ g             (j on partitions, i free)
    P     = exp(S^T)                         (no max-subtraction: |S| <= ~8)
    vt    = x_chunk^T @ wv_h^T               (4096, 64), ones column appended
    acc   = [vt | 1]^T @ P                   -> rows 0..63 = (attn@v)^T unnorm,
                                                row 64     = softmax denoms
    out_h = (wo_h^T)^T @ acc[0:64]           (128, 4096) unnormalized partial
Host: out = sum_h out_h / denom_h  + bias   (the softmax division commutes
with the output projection, so it is applied on the host during unshard).
"""

import numpy as np

HEADS = 8
DIM_HEAD = 64
DIM = 128
HIDDEN = HEADS * DIM_HEAD  # 512
N = 4096  # 16*16*16 tokens
SCALE = DIM_HEAD**-0.5

NI = 512          # query block (free dim of S^T matmuls)
N_IB = N // NI    # 8 i-blocks
JC = 128          # key chunk (partition dim of S^T tiles)
N_JC = N // JC    # 32 j-chunks
JCB = 2           # j-chunks per exp batch (PSUM banks per st tile)

# 'f32' = exact fp32 matmuls (4 cyc/row); 'f32r' = reduced-precision fp32
# matmuls (1 cyc/row at N>=256) for the big attention matmuls.
MM_DTYPE = "f32"

_CACHE = {}


def _build_nc():
    import concourse.bass as bass
    import concourse.mybir as mybir
    import concourse.tile as tile

    f32 = mybir.dt.float32
    Exp = mybir.ActivationFunctionType.Exp

    def mm(ap):
        if MM_DTYPE == "f32r":
            return ap.bitcast(mybir.dt.float32r)
        return ap

    nc = bass.Bass()
    x_d = nc.dram_tensor("x", (DIM, N), f32, kind="ExternalInput")
    b_d = nc.dram_tensor("bmat", (DIM, DIM), f32, kind="ExternalInput")
    wv_d = nc.dram_tensor("wv_t", (DIM, DIM_HEAD), f32, kind="ExternalInput")
    wo_d = nc.dram_tensor("wo_t", (DIM_HEAD, DIM), f32, kind="ExternalInput")
    out_d = nc.dram_tensor("out", (DIM, N), f32, kind="ExternalOutput")
    s_d = nc.dram_tensor("denom", (1, N), f32, kind="ExternalOutput")

    with tile.TileContext(nc) as tc:
        with (
            tc.tile_pool(name="consts", bufs=1) as consts,
            tc.tile_pool(name="data", bufs=1) as data,
            tc.tile_pool(name="ppool", bufs=3) as ppool,
            tc.tile_pool(name="fpool", bufs=2) as fpool,
            tc.tile_pool(name="ps_misc", bufs=2, space="PSUM") as ps_misc,
            tc.tile_pool(name="ps_st", bufs=2, space="PSUM") as ps_st,
            tc.tile_pool(name="ps_acc", bufs=2, space="PSUM") as ps_acc,
        ):
            # ---- load inputs ----
            x_sb = consts.tile([DIM, N], f32)
            nc.sync.dma_start(out=x_sb, in_=x_d[:, :])
            b_sb = consts.tile([DIM, DIM], f32)
            nc.sync.dma_start(out=b_sb, in_=b_d[:, :])
            wv_sb = consts.tile([DIM, DIM_HEAD], f32)
            nc.sync.dma_start(out=wv_sb, in_=wv_d[:, :])
            wo_sb = consts.tile([DIM_HEAD, DIM], f32)
            nc.sync.dma_start(out=wo_sb, in_=wo_d[:, :])

            # Walrus encodes at most ONE sync-wait on an fp32 matmul (inline
            # weight load). Tiny 1x1 "absorber" matmuls make the PE stream
            # observe producer semaphores one at a time so no real matmul
            # ever needs two waits.
            def absorb(read_ap, out_ap, name):
                nc.tensor.matmul(
                    out_ap[0:1, 0:1],
                    lhsT=read_ap[0:1, 0:1],
                    rhs=read_ap[0:1, 0:1],
                    skip_group_check=True,
                )

            # preamble: observe all four input DMAs on PE (small ones first);
            # disjoint columns of one tile so the dummies carry no WAW deps
            pre_ps = ps_misc.tile([1, 4], f32, tag="m", name="pre_ps")
            for k, t in enumerate((b_sb, wv_sb, wo_sb, x_sb)):
                nc.tensor.matmul(
                    pre_ps[0:1, k:k + 1],
                    lhsT=t[0:1, 0:1],
                    rhs=t[0:1, 0:1],
                    skip_group_check=True,
                )

            # ---- g = B^T @ x : (128, 4096) ----
            g_sb = data.tile([DIM, N], f32)
            for ib in range(N_IB):
                isl = slice(ib * NI, (ib + 1) * NI)
                g_ps = ps_misc.tile([DIM, NI], f32, tag="m", name=f"g_ps{ib}")
                nc.tensor.matmul(g_ps, lhsT=b_sb, rhs=x_sb[:, isl])
                nc.vector.tensor_copy(g_sb[:, isl], g_ps)

            # ---- vt chunks: vt[j, d] = v[d, j], with ones column at d=64 ----
            vt_sb = data.tile([JC, N_JC, DIM_HEAD + 1], f32)
            nc.vector.memset(vt_sb[:, :, DIM_HEAD], 1.0)
            for jc in range(N_JC):
                jsl = slice(jc * JC, (jc + 1) * JC)
                vt_ps = ps_misc.tile([JC, DIM_HEAD], f32, tag="m", name=f"vt_ps{jc}")
                nc.tensor.matmul(vt_ps, lhsT=x_sb[:, jsl], rhs=wv_sb)
                nc.vector.tensor_copy(vt_sb[:, jc, 0:DIM_HEAD], vt_ps)

            # ---- attention ----
            # rows 0..63: (attn@v)^T unnormalized; row 64: softmax denoms
            oh_sb = data.tile([DIM_HEAD + 1, N], f32)

            for ib in range(N_IB):
                isl = slice(ib * NI, (ib + 1) * NI)
                # Per-ib DVE-observer: a tiny PE matmul that waits on the
                # newest DVE tick the coming PV matmuls / acc slot need
                # (vt copies for ib 0-1, the acc->oh copy of ib-2 after), so
                # real matmuls never carry a DVE wait on top of an ACT wait.
                obs_src = (vt_sb[:, N_JC - 1, :] if ib < 2
                           else oh_sb[:, (ib - 2) * NI:(ib - 2) * NI + 1])
                obs_ps = ps_misc.tile([1, 1], f32, tag="m", name=f"obs{ib}")
                absorb(obs_src, obs_ps, f"obs{ib}")

                acc_ps = ps_acc.tile([DIM_HEAD + 1, NI], f32, tag="acc",
                                     name=f"acc{ib}")
                for b0 in range(0, N_JC, JCB):
                    nb = min(JCB, N_JC - b0)
                    st_ps = ps_st.tile([JC, JCB * NI], f32, tag="st",
                                       name=f"st{ib}_{b0}")
                    if b0 == 0:
                        # absorb the st-slot WAR wait (vs previous exp read)
                        absorb(x_sb, st_ps, f"st_abs{ib}")
                    for t in range(nb):
                        jc = b0 + t
                        jsl = slice(jc * JC, (jc + 1) * JC)
                        nc.tensor.matmul(
                            st_ps[:, t * NI:(t + 1) * NI],
                            lhsT=mm(x_sb[:, jsl]),
                            rhs=mm(g_sb[:, isl]),
                        )
                    p_sb = ppool.tile([JC, JCB * NI], f32, tag="p",
                                      name=f"p{ib}_{b0}")
                    nc.scalar.activation(
                        out=p_sb[:, : nb * NI],
                        in_=st_ps[:, : nb * NI],
                        func=Exp,
                    )
                    for t in range(nb):
                        jc = b0 + t
                        nc.tensor.matmul(
                            acc_ps,
                            lhsT=mm(vt_sb[:, jc, :]),
                            rhs=mm(p_sb[:, t * NI:(t + 1) * NI]),
                            start=(jc == 0),
                            stop=(jc == N_JC - 1),
                        )
                nc.vector.tensor_copy(oh_sb[:, isl], acc_ps)

            # ---- output projection (normalization happens on host) ----
            for ib in range(N_IB):
                isl = slice(ib * NI, (ib + 1) * NI)
                f_ps = ps_misc.tile([DIM, NI], f32, tag="m", name=f"f_ps{ib}")
                nc.tensor.matmul(f_ps, lhsT=wo_sb, rhs=oh_sb[0:DIM_HEAD, isl])
                f_sb = fpool.tile([DIM, NI], f32, tag="f", name=f"f_sb{ib}")
                # absorb the f_sb slot-release wait (output DMA of ib-2) on a
                # tiny memset so the copy carries a single (PE) wait
                nc.vector.memset(f_sb[0:1, 0:1], 0.0)
                nc.vector.tensor_copy(f_sb, f_ps)
                nc.sync.dma_start(out=out_d[:, isl], in_=f_sb)
            # softmax denominators exit from partition 64 via DMA
            nc.sync.dma_start(out=s_d[:, :], in_=oh_sb[DIM_HEAD:DIM_HEAD + 1, :])

    return nc


# Instruction types whose semaphore updates fire in engine program order
# (compute engines are strict-FIFO; DMA completions are async and excluded).
_INORDER_TYPES = (
    "InstMatmult", "InstLdweights", "InstActivation", "InstTensorCopy",
    "InstTensorTensor", "InstTensorScalarPtr", "InstTensorReduce",
    "InstMemset", "InstReciprocal", "InstPartitionBroadcast", "InstIota",
    "InstBnStats", "InstBnAggr", "InstTensorTensorScan", "InstSelect",
    "InstCustomDveAnt",
)


def _strip_redundant_self_waits(nc):
    """Walrus encodes at most ONE sync-wait per compute instruction. Tile
    emits same-engine WAW/WAR waits that are always satisfied by the
    engine's in-order execution; strip exactly those (and merge same-sem
    duplicates) so every instruction carries <=1 wait."""
    from collections import defaultdict

    f = nc.m.functions[0]
    sem_inc_only = {}
    for blk in f.blocks:
        for inst in blk.instructions:
            si = inst.sync_info
            if si is None:
                continue
            for u in si.on_update:
                ok = u.update_mode == "sem-inc"
                nm = u.ant_name
                sem_inc_only[nm] = sem_inc_only.get(nm, True) and ok

    cum = defaultdict(int)  # (engine, sem) -> incs from in-order instructions
    leftover = []
    for blk in f.blocks:
        for inst in blk.instructions:
            si = inst.sync_info
            if si is None:
                continue
            eng = inst.engine
            tname = type(inst).__name__
            if len(si.on_wait) > 1:
                merged = {}
                for w in si.on_wait:
                    k = w.ant_name
                    if k not in merged or w.wait_value > merged[k].wait_value:
                        merged[k] = w
                self_updates = {u.ant_name for u in si.on_update}
                keep = []
                for w in merged.values():
                    if (w.wait_mode == "sem-ge-imm"
                            and sem_inc_only.get(w.ant_name, False)
                            and cum[(eng, w.ant_name)] >= w.wait_value):
                        continue
                    if (tname == "InstDMACopy" and w.ant_name in self_updates):
                        # same HWDGE queue: descriptors complete in FIFO
                        # order, so waiting on this queue's earlier
                        # descriptor is redundant
                        continue
                    keep.append(w)
                if len(keep) != len(si.on_wait):
                    si.on_wait = keep
                if len(keep) > 1 and tname in _INORDER_TYPES:
                    leftover.append((inst.name, tname,
                                     [(w.ant_name, w.wait_value) for w in keep]))
            if tname in _INORDER_TYPES:
                for u in si.on_update:
                    if u.update_mode == "sem-inc":
                        cum[(eng, u.ant_name)] += u.update_value

    # Second pass: any instruction still carrying >1 wait gets all but one
    # split onto standalone single-wait EventSemaphore instructions inserted
    # just before it on the same engine queue.
    import concourse.mybir as mybir
    nsplit = 0
    for blk in f.blocks:
        insts = list(blk.instructions)
        out = []
        changed = False
        for inst in insts:
            si = inst.sync_info
            if si is not None and len(si.on_wait) > 1:
                waits = list(si.on_wait)
                for i, w in enumerate(waits[:-1]):
                    ev = mybir.InstEventSemaphore(
                        name=f"{inst.name}_sw{i}",
                        engine=inst.engine,
                        ins=[],
                        outs=[],
                        sync_info=mybir.SyncInfo(on_wait=[w], on_update=[]),
                    )
                    out.append(ev)
                    nsplit += 1
                si.on_wait = [waits[-1]]
                changed = True
            out.append(inst)
        if changed:
            blk.instructions = out
    return leftover, nsplit


def _prep_inputs(x, w_qkv, w_out):
    x_flat = np.ascontiguousarray(x.reshape(DIM, N), dtype=np.float32)
    in_maps = []
    for h in range(HEADS):
        wq = w_qkv[h * DIM_HEAD:(h + 1) * DIM_HEAD].astype(np.float64)
        wk = w_qkv[HIDDEN + h * DIM_HEAD:HIDDEN + (h + 1) * DIM_HEAD].astype(
            np.float64)
        wv = w_qkv[2 * HIDDEN + h * DIM_HEAD:2 * HIDDEN + (h + 1) * DIM_HEAD]
        bmat = (SCALE * (wq.T @ wk)).astype(np.float32)
        wv_t = np.ascontiguousarray(wv.T, dtype=np.float32)
        wo_t = np.ascontiguousarray(
            w_out[:, h * DIM_HEAD:(h + 1) * DIM_HEAD].T, dtype=np.float32)
        in_maps.append({
            "x": x_flat,
            "bmat": np.ascontiguousarray(bmat),
            "wv_t": wv_t,
            "wo_t": wo_t,
        })
    return in_maps


def kernel(x, w_qkv, w_out, b_out):
    from concourse.bass_utils import run_bass_kernel_spmd

    if "nc" not in _CACHE:
        nc = _build_nc()
        _strip_redundant_self_waits(nc)
        _CACHE["nc"] = nc
    nc = _CACHE["nc"]

    in_maps = _prep_inputs(np.asarray(x), np.asarray(w_qkv), np.asarray(w_out))
    res = run_bass_kernel_spmd(nc, in_maps, core_ids=list(range(HEADS)))
    total = np.zeros((DIM, N), dtype=np.float64)
    for r in res.results:
        total += r["out"].astype(np.float64) / r["denom"].astype(np.float64)
    total += np.asarray(b_out, dtype=np.float64)[:, None]
    return total.astype(np.float32).reshape(1, DIM, 16, 16, 16)


# revision 21
# speedup vs baseline: 3102.6655x; 3102.6655x over previous
"""Bass/Trainium2 kernel for nn_Attention3D (dense transformer attention over
a 16^3 volume, 8 heads, dim_head 64).

Strategy: head-parallel across the 8 NeuronCores (1 head per core).
Per core (head h):
    B_h   = scale * wq_h^T @ wk_h            (128x128, host-precomputed)
    g     = B_h^T @ x                        (128, 4096)   on device
    S^T   chunks = x_chunk^T @ g             (j on partitions, i free)
    P     = exp(S^T)                         (no max-subtraction: |S| <= ~8)
    vt    = x_chunk^T @ wv_h^T               (4096, 64), ones column appended
    acc   = [vt | 1]^T @ P                   -> rows 0..63 = (attn@v)^T unnorm,
                                                row 64     = softmax denoms
    out_h = (wo_h^T)^T @ acc[0:64]           (128, 4096) unnormalized partial
Host: out = sum_h out_h / denom_h  + bias   (the softmax division commutes
with the output projection, so it is applied on the host during unshard).
"""

import numpy as np

HEADS = 8
DIM_HEAD = 64
DIM = 128
HIDDEN = HEADS * DIM_HEAD  # 512
N = 4096  # 16*16*16 tokens
SCALE = DIM_HEAD**-0.5

NI = 512          # query block (free dim of S^T matmuls)
N_IB = N // NI    # 8 i-blocks
JC = 128          # key chunk (partition dim of S^T tiles)
N_JC = N // JC    # 32 j-chunks
JCB = 2           # j-chunks per exp batch (PSUM banks per st tile)

# 'f32' = exact fp32 matmuls (4 cyc/row); 'f32r' = reduced-precision fp32
# matmuls (1 cyc/row at N>=256) for the big attention matmuls.
MM_DTYPE = "f32"

_CACHE = {}


def _build_nc():
    import concourse.bass as bass
    import concourse.mybir as mybir
    import concourse.tile as tile

    f32 = mybir.dt.float32
    Exp = mybir.ActivationFunctionType.Exp

    def mm(ap):
        if MM_DTYPE == "f32r":
            return ap.bitcast(mybir.dt.float32r)
        return ap

    # producers of f32r-matmul operands must declare f32r output (the BIR
    # verifier requires operands of an FP32r matmult to be rounded)
    mmw = mm

    nc = bass.Bass()
    x_d = nc.dram_tensor("x", (DIM, N), f32, kind="ExternalInput")
    b_d = nc.dram_tensor("bmat", (DIM, DIM), f32, kind="ExternalInput")
    wv_d = nc.dram_tensor("wv_t", (DIM, DIM_HEAD), f32, kind="ExternalInput")
    wo_d = nc.dram_tensor("wo_t", (DIM_HEAD, DIM), f32, kind="ExternalInput")
    ones_d = nc.dram_tensor("ones", (1, N_JC), f32, kind="ExternalInput")
    out_d = nc.dram_tensor("out", (DIM, N), f32, kind="ExternalOutput")
    s_d = nc.dram_tensor("denom", (1, N), f32, kind="ExternalOutput")

    with tile.TileContext(nc) as tc:
        with (
            tc.tile_pool(name="consts", bufs=1) as consts,
            tc.tile_pool(name="data", bufs=1) as data,
            tc.tile_pool(name="ppool", bufs=3) as ppool,
            tc.tile_pool(name="fpool", bufs=2) as fpool,
            tc.tile_pool(name="ps_misc", bufs=2, space="PSUM") as ps_misc,
            tc.tile_pool(name="ps_st", bufs=2, space="PSUM") as ps_st,
            tc.tile_pool(name="ps_acc", bufs=2, space="PSUM") as ps_acc,
        ):
            # ---- load inputs ----
            x_sb = consts.tile([DIM, N], f32)
            nc.sync.dma_start(out=mmw(x_sb), in_=mm(x_d[:, :]))
            b_sb = consts.tile([DIM, DIM], f32)
            nc.sync.dma_start(out=b_sb, in_=b_d[:, :])
            wv_sb = consts.tile([DIM, DIM_HEAD], f32)
            nc.sync.dma_start(out=wv_sb, in_=wv_d[:, :])
            wo_sb = consts.tile([DIM_HEAD, DIM], f32)
            nc.sync.dma_start(out=wo_sb, in_=wo_d[:, :])

            # Walrus encodes at most ONE sync-wait on an fp32 matmul (inline
            # weight load). Tiny 1x1 "absorber" matmuls make the PE stream
            # observe producer semaphores one at a time so no real matmul
            # ever needs two waits.
            def absorb(read_ap, out_ap, name):
                nc.tensor.matmul(
                    out_ap[0:1, 0:1],
                    lhsT=read_ap[0:1, 0:1],
                    rhs=read_ap[0:1, 0:1],
                    skip_group_check=True,
                )

            # preamble: observe all four input DMAs on PE (small ones first);
            # disjoint columns of one tile so the dummies carry no WAW deps
            pre_ps = ps_misc.tile([1, 4], f32, tag="m", name="pre_ps")
            for k, t in enumerate((b_sb, wv_sb, wo_sb, x_sb)):
                nc.tensor.matmul(
                    pre_ps[0:1, k:k + 1],
                    lhsT=t[0:1, 0:1],
                    rhs=t[0:1, 0:1],
                    skip_group_check=True,
                )

            # ---- g = B^T @ x : (128, 4096) ----
            g_sb = data.tile([DIM, N], f32)
            for ib in range(N_IB):
                isl = slice(ib * NI, (ib + 1) * NI)
                g_ps = ps_misc.tile([DIM, NI], f32, tag="m", name=f"g_ps{ib}")
                nc.tensor.matmul(g_ps, lhsT=b_sb, rhs=x_sb[:, isl])
                nc.vector.tensor_copy(mmw(g_sb[:, isl]), g_ps)

            # ---- vt chunks: vt[j, d] = v[d, j], with ones column at d=64 ----
            vt_sb = data.tile([JC, N_JC, DIM_HEAD + 1], f32)
            nc.sync.dma_start(out=mmw(vt_sb[:, :, DIM_HEAD]),
                              in_=mm(ones_d[0:1, :].to_broadcast((JC, N_JC))))
            for jc in range(N_JC):
                jsl = slice(jc * JC, (jc + 1) * JC)
                vt_ps = ps_misc.tile([JC, DIM_HEAD], f32, tag="m", name=f"vt_ps{jc}")
                nc.tensor.matmul(vt_ps, lhsT=x_sb[:, jsl], rhs=wv_sb)
                nc.vector.tensor_copy(mmw(vt_sb[:, jc, 0:DIM_HEAD]), vt_ps)

            # ---- attention ----
            # rows 0..63: (attn@v)^T unnormalized; row 64: softmax denoms
            oh_sb = data.tile([DIM_HEAD + 1, N], f32)

            for ib in range(N_IB):
                isl = slice(ib * NI, (ib + 1) * NI)
                # Per-ib DVE-observer: a tiny PE matmul that waits on the
                # newest DVE tick the coming PV matmuls / acc slot need
                # (vt copies for ib 0-1, the acc->oh copy of ib-2 after), so
                # real matmuls never carry a DVE wait on top of an ACT wait.
                obs_src = (vt_sb[:, N_JC - 1, :] if ib < 2
                           else oh_sb[:, (ib - 2) * NI:(ib - 2) * NI + 1])
                obs_ps = ps_misc.tile([1, 1], f32, tag="m", name=f"obs{ib}")
                absorb(obs_src, obs_ps, f"obs{ib}")

                acc_ps = ps_acc.tile([DIM_HEAD + 1, NI], f32, tag="acc",
                                     name=f"acc{ib}")
                for b0 in range(0, N_JC, JCB):
                    nb = min(JCB, N_JC - b0)
                    st_ps = ps_st.tile([JC, JCB * NI], f32, tag="st",
                                       name=f"st{ib}_{b0}")
                    if b0 == 0:
                        # absorb the st-slot WAR wait (vs previous exp read)
                        absorb(x_sb, st_ps, f"st_abs{ib}")
                    for t in range(nb):
                        jc = b0 + t
                        jsl = slice(jc * JC, (jc + 1) * JC)
                        nc.tensor.matmul(
                            st_ps[:, t * NI:(t + 1) * NI],
                            lhsT=mm(x_sb[:, jsl]),
                            rhs=mm(g_sb[:, isl]),
                        )
                    p_sb = ppool.tile([JC, JCB * NI], f32, tag="p",
                                      name=f"p{ib}_{b0}")
                    nc.scalar.activation(
                        out=mmw(p_sb[:, : nb * NI]),
                        in_=st_ps[:, : nb * NI],
                        func=Exp,
                    )
                    for t in range(nb):
                        jc = b0 + t
                        nc.tensor.matmul(
                            acc_ps,
                            lhsT=mm(vt_sb[:, jc, :]),
                            rhs=mm(p_sb[:, t * NI:(t + 1) * NI]),
                            start=(jc == 0),
                            stop=(jc == N_JC - 1),
                        )
                nc.vector.tensor_copy(oh_sb[:, isl], acc_ps)

            # ---- output projection (normalization happens on host) ----
            for ib in range(N_IB):
                isl = slice(ib * NI, (ib + 1) * NI)
                f_ps = ps_misc.tile([DIM, NI], f32, tag="m", name=f"f_ps{ib}")
                nc.tensor.matmul(f_ps, lhsT=wo_sb, rhs=oh_sb[0:DIM_HEAD, isl])
                f_sb = fpool.tile([DIM, NI], f32, tag="f", name=f"f_sb{ib}")
                # absorb the f_sb slot-release wait (output DMA of ib-2) on a
                # tiny memset so the copy carries a single (PE) wait
                nc.vector.memset(f_sb[0:1, 0:1], 0.0)
                nc.vector.tensor_copy(f_sb, f_ps)
                nc.sync.dma_start(out=out_d[:, isl], in_=f_sb)
            # softmax denominators exit from partition 64 via DMA
            nc.sync.dma_start(out=s_d[:, :], in_=oh_sb[DIM_HEAD:DIM_HEAD + 1, :])

    return nc


# Instruction types whose semaphore updates fire in engine program order
# (compute engines are strict-FIFO; DMA completions are async and excluded).
_INORDER_TYPES = (
    "InstMatmult", "InstLdweights", "InstActivation", "InstTensorCopy",
    "InstTensorTensor", "InstTensorScalarPtr", "InstTensorReduce",
    "InstMemset", "InstReciprocal", "InstPartitionBroadcast", "InstIota",
    "InstBnStats", "InstBnAggr", "InstTensorTensorScan", "InstSelect",
    "InstCustomDveAnt",
)


def _strip_redundant_self_waits(nc):
    """Walrus encodes at most ONE sync-wait per compute instruction. Tile
    emits same-engine WAW/WAR waits that are always satisfied by the
    engine's in-order execution; strip exactly those (and merge same-sem
    duplicates) so every instruction carries <=1 wait."""
    from collections import defaultdict

    f = nc.m.functions[0]
    sem_inc_only = {}
    for blk in f.blocks:
        for inst in blk.instructions:
            si = inst.sync_info
            if si is None:
                continue
            for u in si.on_update:
                ok = u.update_mode == "sem-inc"
                nm = u.ant_name
                sem_inc_only[nm] = sem_inc_only.get(nm, True) and ok

    cum = defaultdict(int)  # (engine, sem) -> incs from in-order instructions
    leftover = []
    for blk in f.blocks:
        for inst in blk.instructions:
            si = inst.sync_info
            if si is None:
                continue
            eng = inst.engine
            tname = type(inst).__name__
            if len(si.on_wait) > 1:
                merged = {}
                for w in si.on_wait:
                    k = w.ant_name
                    if k not in merged or w.wait_value > merged[k].wait_value:
                        merged[k] = w
                self_updates = {u.ant_name for u in si.on_update}
                keep = []
                for w in merged.values():
                    if (w.wait_mode == "sem-ge-imm"
                            and sem_inc_only.get(w.ant_name, False)
                            and cum[(eng, w.ant_name)] >= w.wait_value):
                        continue
                    if (tname == "InstDMACopy" and w.ant_name in self_updates):
                        # same HWDGE queue: descriptors complete in FIFO
                        # order, so waiting on this queue's earlier
                        # descriptor is redundant
                        continue
                    keep.append(w)
                if len(keep) != len(si.on_wait):
                    si.on_wait = keep
                if len(keep) > 1 and tname in _INORDER_TYPES:
                    leftover.append((inst.name, tname,
                                     [(w.ant_name, w.wait_value) for w in keep]))
            if tname in _INORDER_TYPES:
                for u in si.on_update:
                    if u.update_mode == "sem-inc":
                        cum[(eng, u.ant_name)] += u.update_value

    # Second pass: any instruction still carrying >1 wait gets all but one
    # split onto standalone single-wait EventSemaphore instructions inserted
    # just before it on the same engine queue.
    import concourse.mybir as mybir
    nsplit = 0
    for blk in f.blocks:
        insts = list(blk.instructions)
        out = []
        changed = False
        for inst in insts:
            si = inst.sync_info
            if si is not None and len(si.on_wait) > 1:
                waits = list(si.on_wait)
                for i, w in enumerate(waits[:-1]):
                    ev = mybir.InstEventSemaphore(
                        name=f"{inst.name}_sw{i}",
                        engine=inst.engine,
                        ins=[],
                        outs=[],
                        sync_info=mybir.SyncInfo(on_wait=[w], on_update=[]),
                    )
                    out.append(ev)
                    nsplit += 1
                si.on_wait = [waits[-1]]
                changed = True
            out.append(inst)
        if changed:
            blk.instructions = out
    return leftover, nsplit


def _prep_inputs(x, w_qkv, w_out):
    x_flat = np.ascontiguousarray(x.reshape(DIM, N), dtype=np.float32)
    in_maps = []
    for h in range(HEADS):
        wq = w_qkv[h * DIM_HEAD:(h + 1) * DIM_HEAD].astype(np.float64)
        wk = w_qkv[HIDDEN + h * DIM_HEAD:HIDDEN + (h + 1) * DIM_HEAD].astype(
            np.float64)
        wv = w_qkv[2 * HIDDEN + h * DIM_HEAD:2 * HIDDEN + (h + 1) * DIM_HEAD]
        bmat = (SCALE * (wq.T @ wk)).astype(np.float32)
        wv_t = np.ascontiguousarray(wv.T, dtype=np.float32)
        wo_t = np.ascontiguousarray(
            w_out[:, h * DIM_HEAD:(h + 1) * DIM_HEAD].T, dtype=np.float32)
        in_maps.append({
            "x": x_flat,
            "ones": np.ones((1, N_JC), dtype=np.float32),
            "bmat": np.ascontiguousarray(bmat),
            "wv_t": wv_t,
            "wo_t": wo_t,
        })
    return in_maps


def kernel(x, w_qkv, w_out, b_out):
    from concourse.bass_utils import run_bass_kernel_spmd

    if "nc" not in _CACHE:
        nc = _build_nc()
        _strip_redundant_self_waits(nc)
        _CACHE["nc"] = nc
    nc = _CACHE["nc"]

    in_maps = _prep_inputs(np.asarray(x), np.asarray(w_qkv), np.asarray(w_out))
    res = run_bass_kernel_spmd(nc, in_maps, core_ids=list(range(HEADS)))
    total = np.zeros((DIM, N), dtype=np.float64)
    for r in res.results:
        total += r["out"].astype(np.float64) / r["denom"].astype(np.float64)
    total += np.asarray(b_out, dtype=np.float64)[:, None]
    return total.astype(np.float32).reshape(1, DIM, 16, 16, 16)


# revision 22
# speedup vs baseline: 4138.3542x; 1.3338x over previous
"""Bass/Trainium2 kernel for nn_Attention3D (dense transformer attention over
a 16^3 volume, 8 heads, dim_head 64).

Strategy: head-parallel across the 8 NeuronCores (1 head per core).
Per core (head h):
    B_h   = scale * wq_h^T @ wk_h            (128x128, host-precomputed)
    g     = B_h^T @ x                        (128, 4096)   on device
    S^T   chunks = x_chunk^T @ g             (j on partitions, i free)
    P     = exp(S^T)                         (no max-subtraction: |S| <= ~8)
    vt    = x_chunk^T @ wv_h^T               (4096, 64), ones column appended
    acc   = [vt | 1]^T @ P                   -> rows 0..63 = (attn@v)^T unnorm,
                                                row 64     = softmax denoms
    out_h = (wo_h^T)^T @ acc[0:64]           (128, 4096) unnormalized partial
Host: out = sum_h out_h / denom_h  + bias   (the softmax division commutes
with the output projection, so it is applied on the host during unshard).
"""

import numpy as np

HEADS = 8
DIM_HEAD = 64
DIM = 128
HIDDEN = HEADS * DIM_HEAD  # 512
N = 4096  # 16*16*16 tokens
SCALE = DIM_HEAD**-0.5

NI = 512          # query block (free dim of S^T matmuls)
N_IB = N // NI    # 8 i-blocks
JC = 128          # key chunk (partition dim of S^T tiles)
N_JC = N // JC    # 32 j-chunks
JCB = 2           # j-chunks per exp batch (PSUM banks per st tile)

# 'f32' = exact fp32 matmuls (4 cyc/row); 'f32r' = reduced-precision fp32
# matmuls (1 cyc/row at N>=256) for the big attention matmuls.
MM_DTYPE = "f32"

_CACHE = {}


def _build_nc():
    import concourse.bass as bass
    import concourse.mybir as mybir
    import concourse.tile as tile

    f32 = mybir.dt.float32
    Exp = mybir.ActivationFunctionType.Exp

    st_r = MM_DTYPE in ("f32r", "f32r_st")
    pv_r = MM_DTYPE in ("f32r", "f32r_pv")

    def _cast(ap, on):
        return ap.bitcast(mybir.dt.float32r) if on else ap

    def mm_st(ap):
        return _cast(ap, st_r)

    def mm_pv(ap):
        return _cast(ap, pv_r)

    nc = bass.Bass()
    x_d = nc.dram_tensor("x", (DIM, N), f32, kind="ExternalInput")
    b_d = nc.dram_tensor("bmat", (DIM, DIM), f32, kind="ExternalInput")
    wv_d = nc.dram_tensor("wv_t", (DIM, DIM_HEAD), f32, kind="ExternalInput")
    wo_d = nc.dram_tensor("wo_t", (DIM_HEAD, DIM), f32, kind="ExternalInput")
    ones_d = nc.dram_tensor("ones", (1, N_JC), f32, kind="ExternalInput")
    out_d = nc.dram_tensor("out", (DIM, N), f32, kind="ExternalOutput")
    s_d = nc.dram_tensor("denom", (1, N), f32, kind="ExternalOutput")

    with tile.TileContext(nc) as tc:
        with (
            tc.tile_pool(name="consts", bufs=1) as consts,
            tc.tile_pool(name="data", bufs=1) as data,
            tc.tile_pool(name="ppool", bufs=3) as ppool,
            tc.tile_pool(name="fpool", bufs=2) as fpool,
            tc.tile_pool(name="ps_misc", bufs=2, space="PSUM") as ps_misc,
            tc.tile_pool(name="ps_st", bufs=2, space="PSUM") as ps_st,
            tc.tile_pool(name="ps_acc", bufs=2, space="PSUM") as ps_acc,
        ):
            # ---- load inputs ----
            x_sb = consts.tile([DIM, N], f32)
            nc.sync.dma_start(out=mm_st(x_sb), in_=mm_st(x_d[:, :]))
            b_sb = consts.tile([DIM, DIM], f32)
            nc.sync.dma_start(out=b_sb, in_=b_d[:, :])
            wv_sb = consts.tile([DIM, DIM_HEAD], f32)
            nc.sync.dma_start(out=wv_sb, in_=wv_d[:, :])
            wo_sb = consts.tile([DIM_HEAD, DIM], f32)
            nc.sync.dma_start(out=wo_sb, in_=wo_d[:, :])

            # Walrus encodes at most ONE sync-wait on an fp32 matmul (inline
            # weight load). Tiny 1x1 "absorber" matmuls make the PE stream
            # observe producer semaphores one at a time so no real matmul
            # ever needs two waits.
            def absorb(read_ap, out_ap, name):
                nc.tensor.matmul(
                    out_ap[0:1, 0:1],
                    lhsT=read_ap[0:1, 0:1],
                    rhs=read_ap[0:1, 0:1],
                    skip_group_check=True,
                )

            # preamble: observe all four input DMAs on PE (small ones first);
            # disjoint columns of one tile so the dummies carry no WAW deps
            pre_ps = ps_misc.tile([1, 4], f32, tag="m", name="pre_ps")
            for k, t in enumerate((b_sb, wv_sb, wo_sb, x_sb)):
                nc.tensor.matmul(
                    pre_ps[0:1, k:k + 1],
                    lhsT=t[0:1, 0:1],
                    rhs=t[0:1, 0:1],
                    skip_group_check=True,
                )

            # ---- g = B^T @ x : (128, 4096) ----
            g_sb = data.tile([DIM, N], f32)
            for ib in range(N_IB):
                isl = slice(ib * NI, (ib + 1) * NI)
                g_ps = ps_misc.tile([DIM, NI], f32, tag="m", name=f"g_ps{ib}")
                nc.tensor.matmul(g_ps, lhsT=b_sb, rhs=x_sb[:, isl])
                nc.vector.tensor_copy(mm_st(g_sb[:, isl]), g_ps)

            # ---- vt chunks: vt[j, d] = v[d, j], with ones column at d=64 ----
            vt_sb = data.tile([JC, N_JC, DIM_HEAD + 1], f32)
            nc.sync.dma_start(out=mm_pv(vt_sb[:, :, DIM_HEAD]),
                              in_=mm_pv(ones_d[0:1, :].to_broadcast((JC, N_JC))))
            for jc in range(N_JC):
                jsl = slice(jc * JC, (jc + 1) * JC)
                vt_ps = ps_misc.tile([JC, DIM_HEAD], f32, tag="m", name=f"vt_ps{jc}")
                nc.tensor.matmul(vt_ps, lhsT=x_sb[:, jsl], rhs=wv_sb)
                nc.vector.tensor_copy(mm_pv(vt_sb[:, jc, 0:DIM_HEAD]), vt_ps)

            # ---- attention ----
            # rows 0..63: (attn@v)^T unnormalized; row 64: softmax denoms
            oh_sb = data.tile([DIM_HEAD + 1, N], f32)

            for ib in range(N_IB):
                isl = slice(ib * NI, (ib + 1) * NI)
                # Per-ib DVE-observer: a tiny PE matmul that waits on the
                # newest DVE tick the coming PV matmuls / acc slot need
                # (vt copies for ib 0-1, the acc->oh copy of ib-2 after), so
                # real matmuls never carry a DVE wait on top of an ACT wait.
                obs_src = (vt_sb[:, N_JC - 1, :] if ib < 2
                           else oh_sb[:, (ib - 2) * NI:(ib - 2) * NI + 1])
                obs_ps = ps_misc.tile([1, 1], f32, tag="m", name=f"obs{ib}")
                absorb(obs_src, obs_ps, f"obs{ib}")

                acc_ps = ps_acc.tile([DIM_HEAD + 1, NI], f32, tag="acc",
                                     name=f"acc{ib}")
                for b0 in range(0, N_JC, JCB):
                    nb = min(JCB, N_JC - b0)
                    st_ps = ps_st.tile([JC, JCB * NI], f32, tag="st",
                                       name=f"st{ib}_{b0}")
                    if b0 == 0:
                        # absorb the st-slot WAR wait (vs previous exp read)
                        absorb(x_sb, st_ps, f"st_abs{ib}")
                    for t in range(nb):
                        jc = b0 + t
                        jsl = slice(jc * JC, (jc + 1) * JC)
                        nc.tensor.matmul(
                            st_ps[:, t * NI:(t + 1) * NI],
                            lhsT=mm_st(x_sb[:, jsl]),
                            rhs=mm_st(g_sb[:, isl]),
                        )
                    p_sb = ppool.tile([JC, JCB * NI], f32, tag="p",
                                      name=f"p{ib}_{b0}")
                    nc.scalar.activation(
                        out=mm_pv(p_sb[:, : nb * NI]),
                        in_=st_ps[:, : nb * NI],
                        func=Exp,
                    )
                    for t in range(nb):
                        jc = b0 + t
                        nc.tensor.matmul(
                            acc_ps,
                            lhsT=mm_pv(vt_sb[:, jc, :]),
                            rhs=mm_pv(p_sb[:, t * NI:(t + 1) * NI]),
                            start=(jc == 0),
                            stop=(jc == N_JC - 1),
                        )
                nc.vector.tensor_copy(oh_sb[:, isl], acc_ps)

            # ---- output projection (normalization happens on host) ----
            for ib in range(N_IB):
                isl = slice(ib * NI, (ib + 1) * NI)
                f_ps = ps_misc.tile([DIM, NI], f32, tag="m", name=f"f_ps{ib}")
                nc.tensor.matmul(f_ps, lhsT=wo_sb, rhs=oh_sb[0:DIM_HEAD, isl])
                f_sb = fpool.tile([DIM, NI], f32, tag="f", name=f"f_sb{ib}")
                # absorb the f_sb slot-release wait (output DMA of ib-2) on a
                # tiny memset so the copy carries a single (PE) wait
                nc.vector.memset(f_sb[0:1, 0:1], 0.0)
                nc.vector.tensor_copy(f_sb, f_ps)
                nc.sync.dma_start(out=out_d[:, isl], in_=f_sb)
            # softmax denominators exit from partition 64 via DMA
            nc.sync.dma_start(out=s_d[:, :], in_=oh_sb[DIM_HEAD:DIM_HEAD + 1, :])

    return nc


# Instruction types whose semaphore updates fire in engine program order
# (compute engines are strict-FIFO; DMA completions are async and excluded).
_INORDER_TYPES = (
    "InstMatmult", "InstLdweights", "InstActivation", "InstTensorCopy",
    "InstTensorTensor", "InstTensorScalarPtr", "InstTensorReduce",
    "InstMemset", "InstReciprocal", "InstPartitionBroadcast", "InstIota",
    "InstBnStats", "InstBnAggr", "InstTensorTensorScan", "InstSelect",
    "InstCustomDveAnt",
)


def _strip_redundant_self_waits(nc):
    """Walrus encodes at most ONE sync-wait per compute instruction. Tile
    emits same-engine WAW/WAR waits that are always satisfied by the
    engine's in-order execution; strip exactly those (and merge same-sem
    duplicates) so every instruction carries <=1 wait."""
    from collections import defaultdict

    f = nc.m.functions[0]
    sem_inc_only = {}
    for blk in f.blocks:
        for inst in blk.instructions:
            si = inst.sync_info
            if si is None:
                continue
            for u in si.on_update:
                ok = u.update_mode == "sem-inc"
                nm = u.ant_name
                sem_inc_only[nm] = sem_inc_only.get(nm, True) and ok

    cum = defaultdict(int)  # (engine, sem) -> incs from in-order instructions
    leftover = []
    for blk in f.blocks:
        for inst in blk.instructions:
            si = inst.sync_info
            if si is None:
                continue
            eng = inst.engine
            tname = type(inst).__name__
            if len(si.on_wait) > 1:
                merged = {}
                for w in si.on_wait:
                    k = w.ant_name
                    if k not in merged or w.wait_value > merged[k].wait_value:
                        merged[k] = w
                self_updates = {u.ant_name for u in si.on_update}
                keep = []
                for w in merged.values():
                    if (w.wait_mode == "sem-ge-imm"
                            and sem_inc_only.get(w.ant_name, False)
                            and cum[(eng, w.ant_name)] >= w.wait_value):
                        continue
                    if (tname == "InstDMACopy" and w.ant_name in self_updates):
                        # same HWDGE queue: descriptors complete in FIFO
                        # order, so waiting on this queue's earlier
                        # descriptor is redundant
                        continue
                    keep.append(w)
                if len(keep) != len(si.on_wait):
                    si.on_wait = keep
                if len(keep) > 1 and tname in _INORDER_TYPES:
                    leftover.append((inst.name, tname,
                                     [(w.ant_name, w.wait_value) for w in keep]))
            if tname in _INORDER_TYPES:
                for u in si.on_update:
                    if u.update_mode == "sem-inc":
                        cum[(eng, u.ant_name)] += u.update_value

    # Second pass: any instruction still carrying >1 wait gets all but one
    # split onto standalone single-wait EventSemaphore instructions inserted
    # just before it on the same engine queue.
    import concourse.mybir as mybir
    nsplit = 0
    for blk in f.blocks:
        insts = list(blk.instructions)
        out = []
        changed = False
        for inst in insts:
            si = inst.sync_info
            if si is not None and len(si.on_wait) > 1:
                waits = list(si.on_wait)
                for i, w in enumerate(waits[:-1]):
                    ev = mybir.InstEventSemaphore(
                        name=f"{inst.name}_sw{i}",
                        engine=inst.engine,
                        ins=[],
                        outs=[],
                        sync_info=mybir.SyncInfo(on_wait=[w], on_update=[]),
                    )
                    out.append(ev)
                    nsplit += 1
                si.on_wait = [waits[-1]]
                changed = True
            out.append(inst)
        if changed:
            blk.instructions = out
    return leftover, nsplit


def _prep_inputs(x, w_qkv, w_out):
    x_flat = np.ascontiguousarray(x.reshape(DIM, N), dtype=np.float32)
    in_maps = []
    for h in range(HEADS):
        wq = w_qkv[h * DIM_HEAD:(h + 1) * DIM_HEAD].astype(np.float64)
        wk = w_qkv[HIDDEN + h * DIM_HEAD:HIDDEN + (h + 1) * DIM_HEAD].astype(
            np.float64)
        wv = w_qkv[2 * HIDDEN + h * DIM_HEAD:2 * HIDDEN + (h + 1) * DIM_HEAD]
        bmat = (SCALE * (wq.T @ wk)).astype(np.float32)
        wv_t = np.ascontiguousarray(wv.T, dtype=np.float32)
        wo_t = np.ascontiguousarray(
            w_out[:, h * DIM_HEAD:(h + 1) * DIM_HEAD].T, dtype=np.float32)
        in_maps.append({
            "x": x_flat,
            "ones": np.ones((1, N_JC), dtype=np.float32),
            "bmat": np.ascontiguousarray(bmat),
            "wv_t": wv_t,
            "wo_t": wo_t,
        })
    return in_maps


def kernel(x, w_qkv, w_out, b_out):
    from concourse.bass_utils import run_bass_kernel_spmd

    if "nc" not in _CACHE:
        nc = _build_nc()
        _strip_redundant_self_waits(nc)
        _CACHE["nc"] = nc
    nc = _CACHE["nc"]

    in_maps = _prep_inputs(np.asarray(x), np.asarray(w_qkv), np.asarray(w_out))
    res = run_bass_kernel_spmd(nc, in_maps, core_ids=list(range(HEADS)))
    total = np.zeros((DIM, N), dtype=np.float64)
    for r in res.results:
        total += r["out"].astype(np.float64) / r["denom"].astype(np.float64)
    total += np.asarray(b_out, dtype=np.float64)[:, None]
    return total.astype(np.float32).reshape(1, DIM, 16, 16, 16)


# revision 23
# speedup vs baseline: 8555.9278x; 2.0675x over previous
"""Bass/Trainium2 kernel for nn_Attention3D (dense transformer attention over
a 16^3 volume, 8 heads, dim_head 64).

Strategy: head-parallel across the 8 NeuronCores (1 head per core).
Per core (head h):
    B_h   = scale * wq_h^T @ wk_h            (128x128, host-precomputed)
    g     = B_h^T @ x                        (128, 4096)   on device
    S^T   chunks = x_chunk^T @ g             (j on partitions, i free)
    P     = exp(S^T)                         (no max-subtraction: |S| <= ~8)
    vt    = x_chunk^T @ wv_h^T               (4096, 64), ones column appended
    acc   = [vt | 1]^T @ P                   -> rows 0..63 = (attn@v)^T unnorm,
                                                row 64     = softmax denoms
    out_h = (wo_h^T)^T @ acc[0:64]           (128, 4096) unnormalized partial
Host: out = sum_h out_h / denom_h  + bias   (the softmax division commutes
with the output projection, so it is applied on the host during unshard).
"""

import numpy as np

HEADS = 8
DIM_HEAD = 64
DIM = 128
HIDDEN = HEADS * DIM_HEAD  # 512
N = 4096  # 16*16*16 tokens
SCALE = DIM_HEAD**-0.5

NI = 512          # query block (free dim of S^T matmuls)
N_IB = N // NI    # 8 i-blocks
JC = 128          # key chunk (partition dim of S^T tiles)
N_JC = N // JC    # 32 j-chunks
JCB = 2           # j-chunks per exp batch (PSUM banks per st tile)

# 'f32' = exact fp32 matmuls (4 cyc/row); 'f32r' = reduced-precision fp32
# matmuls (1 cyc/row at N>=256) for the big attention matmuls.
MM_DTYPE = "f32"

_CACHE = {}


def _build_nc():
    import concourse.bass as bass
    import concourse.mybir as mybir
    import concourse.tile as tile

    f32 = mybir.dt.float32
    Exp = mybir.ActivationFunctionType.Exp

    st_r = MM_DTYPE in ("f32r", "f32r_st", "mix")
    pv_r = MM_DTYPE in ("f32r", "f32r_pv")
    pv_bf16 = MM_DTYPE == "mix"
    bf16 = mybir.dt.bfloat16
    pv_dt = bf16 if pv_bf16 else f32

    def _cast(ap, on):
        return ap.bitcast(mybir.dt.float32r) if on else ap

    def mm_st(ap):
        return _cast(ap, st_r)

    def mm_pv(ap):
        return _cast(ap, pv_r)

    nc = bass.Bass()
    x_d = nc.dram_tensor("x", (DIM, N), f32, kind="ExternalInput")
    b_d = nc.dram_tensor("bmat", (DIM, DIM), f32, kind="ExternalInput")
    wv_d = nc.dram_tensor("wv_t", (DIM, DIM_HEAD), f32, kind="ExternalInput")
    wo_d = nc.dram_tensor("wo_t", (DIM_HEAD, DIM), f32, kind="ExternalInput")
    ones_dt = mybir.dt.bfloat16 if MM_DTYPE == "mix" else mybir.dt.float32
    ones_d = nc.dram_tensor("ones", (1, N_JC), ones_dt, kind="ExternalInput")
    out_d = nc.dram_tensor("out", (DIM, N), f32, kind="ExternalOutput")
    s_d = nc.dram_tensor("denom", (1, N), f32, kind="ExternalOutput")

    with tile.TileContext(nc) as tc:
        with (
            tc.tile_pool(name="consts", bufs=1) as consts,
            tc.tile_pool(name="data", bufs=1) as data,
            tc.tile_pool(name="ppool", bufs=3) as ppool,
            tc.tile_pool(name="fpool", bufs=2) as fpool,
            tc.tile_pool(name="ps_misc", bufs=2, space="PSUM") as ps_misc,
            tc.tile_pool(name="ps_st", bufs=2, space="PSUM") as ps_st,
            tc.tile_pool(name="ps_acc", bufs=2, space="PSUM") as ps_acc,
        ):
            # ---- load inputs ----
            x_sb = consts.tile([DIM, N], f32)
            nc.sync.dma_start(out=mm_st(x_sb), in_=mm_st(x_d[:, :]))
            b_sb = consts.tile([DIM, DIM], f32)
            nc.sync.dma_start(out=b_sb, in_=b_d[:, :])
            wv_sb = consts.tile([DIM, DIM_HEAD], f32)
            nc.sync.dma_start(out=wv_sb, in_=wv_d[:, :])
            wo_sb = consts.tile([DIM_HEAD, DIM], f32)
            nc.sync.dma_start(out=wo_sb, in_=wo_d[:, :])

            # Walrus encodes at most ONE sync-wait on an fp32 matmul (inline
            # weight load). Tiny 1x1 "absorber" matmuls make the PE stream
            # observe producer semaphores one at a time so no real matmul
            # ever needs two waits.
            def absorb(read_ap, out_ap, name):
                nc.tensor.matmul(
                    out_ap[0:1, 0:1],
                    lhsT=read_ap[0:1, 0:1],
                    rhs=read_ap[0:1, 0:1],
                    skip_group_check=True,
                )

            # preamble: observe all four input DMAs on PE (small ones first);
            # disjoint columns of one tile so the dummies carry no WAW deps
            pre_ps = ps_misc.tile([1, 4], f32, tag="m", name="pre_ps")
            for k, t in enumerate((b_sb, wv_sb, wo_sb, x_sb)):
                nc.tensor.matmul(
                    pre_ps[0:1, k:k + 1],
                    lhsT=t[0:1, 0:1],
                    rhs=t[0:1, 0:1],
                    skip_group_check=True,
                )

            # ---- g = B^T @ x : (128, 4096) ----
            g_sb = data.tile([DIM, N], f32)
            for ib in range(N_IB):
                isl = slice(ib * NI, (ib + 1) * NI)
                g_ps = ps_misc.tile([DIM, NI], f32, tag="m", name=f"g_ps{ib}")
                nc.tensor.matmul(g_ps, lhsT=b_sb, rhs=x_sb[:, isl])
                nc.vector.tensor_copy(mm_st(g_sb[:, isl]), g_ps)

            # ---- vt chunks: vt[j, d] = v[d, j], with ones column at d=64 ----
            vt_sb = data.tile([JC, N_JC, DIM_HEAD + 1], pv_dt)
            nc.sync.dma_start(out=mm_pv(vt_sb[:, :, DIM_HEAD]),
                              in_=mm_pv(ones_d[0:1, :].to_broadcast((JC, N_JC))))
            for jc in range(N_JC):
                jsl = slice(jc * JC, (jc + 1) * JC)
                vt_ps = ps_misc.tile([JC, DIM_HEAD], f32, tag="m", name=f"vt_ps{jc}")
                nc.tensor.matmul(vt_ps, lhsT=x_sb[:, jsl], rhs=wv_sb)
                nc.vector.tensor_copy(mm_pv(vt_sb[:, jc, 0:DIM_HEAD]), vt_ps)

            # ---- attention ----
            # rows 0..63: (attn@v)^T unnormalized; row 64: softmax denoms
            oh_sb = data.tile([DIM_HEAD + 1, N], f32)

            for ib in range(N_IB):
                isl = slice(ib * NI, (ib + 1) * NI)
                # Per-ib DVE-observer: a tiny PE matmul that waits on the
                # newest DVE tick the coming PV matmuls / acc slot need
                # (vt copies for ib 0-1, the acc->oh copy of ib-2 after), so
                # real matmuls never carry a DVE wait on top of an ACT wait.
                obs_src = (vt_sb[:, N_JC - 1, :] if ib < 2
                           else oh_sb[:, (ib - 2) * NI:(ib - 2) * NI + 1])
                obs_ps = ps_misc.tile([1, 1], f32, tag="m", name=f"obs{ib}")
                absorb(obs_src, obs_ps, f"obs{ib}")

                acc_ps = ps_acc.tile([DIM_HEAD + 1, NI], f32, tag="acc",
                                     name=f"acc{ib}")
                for b0 in range(0, N_JC, JCB):
                    nb = min(JCB, N_JC - b0)
                    st_ps = ps_st.tile([JC, JCB * NI], f32, tag="st",
                                       name=f"st{ib}_{b0}")
                    if b0 == 0:
                        # absorb the st-slot WAR wait (vs previous exp read)
                        absorb(x_sb, st_ps, f"st_abs{ib}")
                    for t in range(nb):
                        jc = b0 + t
                        jsl = slice(jc * JC, (jc + 1) * JC)
                        nc.tensor.matmul(
                            st_ps[:, t * NI:(t + 1) * NI],
                            lhsT=mm_st(x_sb[:, jsl]),
                            rhs=mm_st(g_sb[:, isl]),
                        )
                    p_sb = ppool.tile([JC, JCB * NI], pv_dt, tag="p",
                                      name=f"p{ib}_{b0}")
                    nc.scalar.activation(
                        out=mm_pv(p_sb[:, : nb * NI]),
                        in_=st_ps[:, : nb * NI],
                        func=Exp,
                    )
                    for t in range(nb):
                        jc = b0 + t
                        nc.tensor.matmul(
                            acc_ps,
                            lhsT=mm_pv(vt_sb[:, jc, :]),
                            rhs=mm_pv(p_sb[:, t * NI:(t + 1) * NI]),
                            start=(jc == 0),
                            stop=(jc == N_JC - 1),
                        )
                nc.vector.tensor_copy(oh_sb[:, isl], acc_ps)

            # ---- output projection (normalization happens on host) ----
            for ib in range(N_IB):
                isl = slice(ib * NI, (ib + 1) * NI)
                f_ps = ps_misc.tile([DIM, NI], f32, tag="m", name=f"f_ps{ib}")
                nc.tensor.matmul(f_ps, lhsT=wo_sb, rhs=oh_sb[0:DIM_HEAD, isl])
                f_sb = fpool.tile([DIM, NI], f32, tag="f", name=f"f_sb{ib}")
                # absorb the f_sb slot-release wait (output DMA of ib-2) on a
                # tiny memset so the copy carries a single (PE) wait
                nc.vector.memset(f_sb[0:1, 0:1], 0.0)
                nc.vector.tensor_copy(f_sb, f_ps)
                nc.sync.dma_start(out=out_d[:, isl], in_=f_sb)
            # softmax denominators exit from partition 64 via DMA
            nc.sync.dma_start(out=s_d[:, :], in_=oh_sb[DIM_HEAD:DIM_HEAD + 1, :])

    return nc


# Instruction types whose semaphore updates fire in engine program order
# (compute engines are strict-FIFO; DMA completions are async and excluded).
_INORDER_TYPES = (
    "InstMatmult", "InstLdweights", "InstActivation", "InstTensorCopy",
    "InstTensorTensor", "InstTensorScalarPtr", "InstTensorReduce",
    "InstMemset", "InstReciprocal", "InstPartitionBroadcast", "InstIota",
    "InstBnStats", "InstBnAggr", "InstTensorTensorScan", "InstSelect",
    "InstCustomDveAnt",
)


def _strip_redundant_self_waits(nc):
    """Walrus encodes at most ONE sync-wait per compute instruction. Tile
    emits same-engine WAW/WAR waits that are always satisfied by the
    engine's in-order execution; strip exactly those (and merge same-sem
    duplicates) so every instruction carries <=1 wait."""
    from collections import defaultdict

    f = nc.m.functions[0]
    sem_inc_only = {}
    for blk in f.blocks:
        for inst in blk.instructions:
            si = inst.sync_info
            if si is None:
                continue
            for u in si.on_update:
                ok = u.update_mode == "sem-inc"
                nm = u.ant_name
                sem_inc_only[nm] = sem_inc_only.get(nm, True) and ok

    cum = defaultdict(int)  # (engine, sem) -> incs from in-order instructions
    leftover = []
    for blk in f.blocks:
        for inst in blk.instructions:
            si = inst.sync_info
            if si is None:
                continue
            eng = inst.engine
            tname = type(inst).__name__
            if len(si.on_wait) > 1:
                merged = {}
                for w in si.on_wait:
                    k = w.ant_name
                    if k not in merged or w.wait_value > merged[k].wait_value:
                        merged[k] = w
                self_updates = {u.ant_name for u in si.on_update}
                keep = []
                for w in merged.values():
                    if (w.wait_mode == "sem-ge-imm"
                            and sem_inc_only.get(w.ant_name, False)
                            and cum[(eng, w.ant_name)] >= w.wait_value):
                        continue
                    if (tname == "InstDMACopy" and w.ant_name in self_updates):
                        # same HWDGE queue: descriptors complete in FIFO
                        # order, so waiting on this queue's earlier
                        # descriptor is redundant
                        continue
                    keep.append(w)
                if len(keep) != len(si.on_wait):
                    si.on_wait = keep
                if len(keep) > 1 and tname in _INORDER_TYPES:
                    leftover.append((inst.name, tname,
                                     [(w.ant_name, w.wait_value) for w in keep]))
            if tname in _INORDER_TYPES:
                for u in si.on_update:
                    if u.update_mode == "sem-inc":
                        cum[(eng, u.ant_name)] += u.update_value

    # Second pass: any instruction still carrying >1 wait gets all but one
    # split onto standalone single-wait EventSemaphore instructions inserted
    # just before it on the same engine queue.
    import concourse.mybir as mybir
    nsplit = 0
    for blk in f.blocks:
        insts = list(blk.instructions)
        out = []
        changed = False
        for inst in insts:
            si = inst.sync_info
            if si is not None and len(si.on_wait) > 1:
                waits = list(si.on_wait)
                for i, w in enumerate(waits[:-1]):
                    ev = mybir.InstEventSemaphore(
                        name=f"{inst.name}_sw{i}",
                        engine=inst.engine,
                        ins=[],
                        outs=[],
                        sync_info=mybir.SyncInfo(on_wait=[w], on_update=[]),
                    )
                    out.append(ev)
                    nsplit += 1
                si.on_wait = [waits[-1]]
                changed = True
            out.append(inst)
        if changed:
            blk.instructions = out
    return leftover, nsplit


def _ones_arr():
    if MM_DTYPE == "mix":
        import ml_dtypes
        return np.ones((1, N_JC), dtype=ml_dtypes.bfloat16)
    return np.ones((1, N_JC), dtype=np.float32)


def _prep_inputs(x, w_qkv, w_out):
    x_flat = np.ascontiguousarray(x.reshape(DIM, N), dtype=np.float32)
    in_maps = []
    for h in range(HEADS):
        wq = w_qkv[h * DIM_HEAD:(h + 1) * DIM_HEAD].astype(np.float64)
        wk = w_qkv[HIDDEN + h * DIM_HEAD:HIDDEN + (h + 1) * DIM_HEAD].astype(
            np.float64)
        wv = w_qkv[2 * HIDDEN + h * DIM_HEAD:2 * HIDDEN + (h + 1) * DIM_HEAD]
        bmat = (SCALE * (wq.T @ wk)).astype(np.float32)
        wv_t = np.ascontiguousarray(wv.T, dtype=np.float32)
        wo_t = np.ascontiguousarray(
            w_out[:, h * DIM_HEAD:(h + 1) * DIM_HEAD].T, dtype=np.float32)
        in_maps.append({
            "x": x_flat,
            "ones": _ones_arr(),
            "bmat": np.ascontiguousarray(bmat),
            "wv_t": wv_t,
            "wo_t": wo_t,
        })
    return in_maps


def kernel(x, w_qkv, w_out, b_out):
    from concourse.bass_utils import run_bass_kernel_spmd

    if "nc" not in _CACHE:
        nc = _build_nc()
        _strip_redundant_self_waits(nc)
        _CACHE["nc"] = nc
    nc = _CACHE["nc"]

    in_maps = _prep_inputs(np.asarray(x), np.asarray(w_qkv), np.asarray(w_out))
    res = run_bass_kernel_spmd(nc, in_maps, core_ids=list(range(HEADS)))
    total = np.zeros((DIM, N), dtype=np.float64)
    for r in res.results:
        total += r["out"].astype(np.float64) / r["denom"].astype(np.float64)
    total += np.asarray(b_out, dtype=np.float64)[:, None]
    return total.astype(np.float32).reshape(1, DIM, 16, 16, 16)


# revision 29
# speedup vs baseline: 9417.4264x; 1.1007x over previous
"""Bass/Trainium2 kernel for nn_Attention3D (dense transformer attention over
a 16^3 volume, 8 heads, dim_head 64).

Strategy: head-parallel across the 8 NeuronCores (1 head per core).
Per core (head h):
    B_h   = scale * wq_h^T @ wk_h            (128x128, host-precomputed)
    g     = B_h^T @ x                        (128, 4096)   on device
    S^T   chunks = x_chunk^T @ g             (j on partitions, i free)
    P     = exp(S^T)                         (no max-subtraction: |S| <= ~8)
    vt    = x_chunk^T @ wv_h^T               (4096, 64), ones column appended
    acc   = [vt | 1]^T @ P                   -> rows 0..63 = (attn@v)^T unnorm,
                                                row 64     = softmax denoms
    out_h = (wo_h^T)^T @ acc[0:64]           (128, 4096) unnormalized partial
Host: out = sum_h out_h / denom_h  + bias   (the softmax division commutes
with the output projection, so it is applied on the host during unshard).
"""

import numpy as np

HEADS = 8
DIM_HEAD = 64
DIM = 128
HIDDEN = HEADS * DIM_HEAD  # 512
N = 4096  # 16*16*16 tokens
SCALE = DIM_HEAD**-0.5

NI = 512          # query block (free dim of S^T matmuls)
N_IB = N // NI    # 8 i-blocks
JC = 128          # key chunk (partition dim of S^T tiles)
N_JC = N // JC    # 32 j-chunks
JCB = 3           # j-chunks per exp batch (PSUM banks per st tile)

# 'f32' = exact fp32 matmuls (4 cyc/row); 'f32r' = reduced-precision fp32
# matmuls (1 cyc/row at N>=256) for the big attention matmuls.
MM_DTYPE = "mix"

_CACHE = {}


def _build_nc():
    import concourse.bass as bass
    import concourse.mybir as mybir
    import concourse.tile as tile

    f32 = mybir.dt.float32
    Exp = mybir.ActivationFunctionType.Exp

    st_r = MM_DTYPE in ("f32r", "f32r_st", "mix")
    pv_r = MM_DTYPE in ("f32r", "f32r_pv")
    pv_bf16 = MM_DTYPE == "mix"
    bf16 = mybir.dt.bfloat16
    pv_dt = bf16 if pv_bf16 else f32

    def _cast(ap, on):
        return ap.bitcast(mybir.dt.float32r) if on else ap

    def mm_st(ap):
        return _cast(ap, st_r)

    def mm_pv(ap):
        return _cast(ap, pv_r)

    nc = bass.Bass()
    x_d = nc.dram_tensor("x", (DIM, N), f32, kind="ExternalInput")
    b_d = nc.dram_tensor("bmat", (DIM, DIM), f32, kind="ExternalInput")
    wv_d = nc.dram_tensor("wv_t", (DIM, DIM_HEAD), f32, kind="ExternalInput")
    wo_d = nc.dram_tensor("wo_t", (DIM_HEAD, DIM), f32, kind="ExternalInput")
    ones_dt = mybir.dt.bfloat16 if MM_DTYPE == "mix" else mybir.dt.float32
    ones_d = nc.dram_tensor("ones", (1, N_JC), ones_dt, kind="ExternalInput")
    out_d = nc.dram_tensor("out", (DIM, N), f32, kind="ExternalOutput")
    s_d = nc.dram_tensor("denom", (1, N), f32, kind="ExternalOutput")

    with tile.TileContext(nc) as tc:
        with (
            tc.tile_pool(name="consts", bufs=1) as consts,
            tc.tile_pool(name="data", bufs=1) as data,
            tc.tile_pool(name="ppool", bufs=3) as ppool,
            tc.tile_pool(name="fpool", bufs=2) as fpool,
            tc.tile_pool(name="ps_st", bufs=2, space="PSUM") as ps_st,
            tc.tile_pool(name="ps_acc", bufs=2, space="PSUM") as ps_acc,
        ):
            # ---- load inputs (small tensors first: the DMA path is
            # near-serial, and b gates the first g matmul) ----
            b_sb = consts.tile([DIM, DIM], f32)
            nc.sync.dma_start(out=mm_st(b_sb), in_=mm_st(b_d[:, :]))
            wv_sb = consts.tile([DIM, DIM_HEAD], f32)
            nc.sync.dma_start(out=wv_sb, in_=wv_d[:, :])
            wo_sb = consts.tile([DIM_HEAD, DIM], f32)
            nc.sync.dma_start(out=mm_st(wo_sb), in_=mm_st(wo_d[:, :]))
            x_sb = consts.tile([DIM, N], f32)
            for ib in range(N_IB):
                xsl = slice(ib * NI, (ib + 1) * NI)
                nc.sync.dma_start(out=mm_st(x_sb[:, xsl]),
                                  in_=mm_st(x_d[:, xsl]))

            # ---- attention (g and vt produced just-in-time inside the
            # pipeline; per-ib epilogue software-pipelined into the next ib's
            # first batch so PE/ACT never drain) ----
            g_sb = data.tile([DIM, N], f32)
            vt_sb = data.tile([JC, N_JC, DIM_HEAD + 1], pv_dt)
            nc.sync.dma_start(out=mm_pv(vt_sb[:, :, DIM_HEAD]),
                              in_=mm_pv(ones_d[0:1, :].to_broadcast((JC, N_JC))))
            oh_sb = data.tile([DIM_HEAD, N], f32)
            s65_sb = data.tile([DIM_HEAD + 1, N], f32)  # row 64 only

            def emit_g(ib):
                isl = slice(ib * NI, (ib + 1) * NI)
                g_ps = ps_acc.tile([DIM, NI], f32, tag="acc",
                                   name=f"g_ps{ib}")
                nc.tensor.matmul(g_ps, lhsT=mm_st(b_sb),
                                 rhs=mm_st(x_sb[:, isl]))
                nc.vector.tensor_copy(mm_st(g_sb[:, isl]), g_ps)

            def emit_vt(jc):
                jsl = slice(jc * JC, (jc + 1) * JC)
                vt_ps = ps_acc.tile([JC, DIM_HEAD], f32, tag="acc",
                                    name=f"vt_ps{jc}")
                nc.tensor.matmul(vt_ps, lhsT=x_sb[:, jsl], rhs=wv_sb)
                nc.vector.tensor_copy(mm_pv(vt_sb[:, jc, 0:DIM_HEAD]), vt_ps)

            def emit_epilogue(ib):
                isl = slice(ib * NI, (ib + 1) * NI)
                acc_ps = accs[ib]
                nc.vector.tensor_copy(mm_st(oh_sb[:, isl]),
                                      acc_ps[0:DIM_HEAD, :])
                nc.vector.tensor_copy(
                    s65_sb[DIM_HEAD:DIM_HEAD + 1, isl],
                    acc_ps[DIM_HEAD:DIM_HEAD + 1, :])
                # output projection for this i-block (normalization on host)
                f_ps = ps_st.tile([DIM, NI], f32, tag="st", name=f"f_ps{ib}")
                nc.tensor.matmul(f_ps, lhsT=mm_st(wo_sb),
                                 rhs=mm_st(oh_sb[:, isl]))
                f_sb = fpool.tile([DIM, NI], f32, tag="f", name=f"f_sb{ib}")
                nc.vector.tensor_copy(f_sb, f_ps)
                nc.sync.dma_start(out=out_d[:, isl], in_=f_sb)

            emit_g(0)
            accs = {}
            batches = [(ib, b0) for ib in range(N_IB)
                       for b0 in range(0, N_JC, JCB)]
            last_b0 = batches[-1][1]

            def emit_exp_pv(ib, b0, st_ps):
                nb = min(JCB, N_JC - b0)
                p_sb = ppool.tile([JC, JCB * NI], pv_dt, tag="p",
                                  name=f"p{ib}_{b0}")
                nc.scalar.activation(
                    out=mm_pv(p_sb[:, : nb * NI]),
                    in_=st_ps[:, : nb * NI],
                    func=Exp,
                )
                for t in range(nb):
                    jc = b0 + t
                    nc.tensor.matmul(
                        accs[ib],
                        lhsT=mm_pv(vt_sb[:, jc, :]),
                        rhs=mm_pv(p_sb[:, t * NI:(t + 1) * NI]),
                        start=(jc == 0),
                        stop=(jc == N_JC - 1),
                    )
                if b0 == last_b0:
                    emit_epilogue(ib)

            pending = None
            for ib, b0 in batches:
                isl = slice(ib * NI, (ib + 1) * NI)
                if b0 == 0:
                    accs[ib] = ps_acc.tile([DIM_HEAD + 1, NI], f32, tag="acc",
                                           name=f"acc{ib}")
                nb = min(JCB, N_JC - b0)
                st_ps = ps_st.tile([JC, JCB * NI], f32, tag="st",
                                   name=f"st{ib}_{b0}")
                for t in range(nb):
                    jc = b0 + t
                    jsl = slice(jc * JC, (jc + 1) * JC)
                    nc.tensor.matmul(
                        st_ps[:, t * NI:(t + 1) * NI],
                        lhsT=mm_st(x_sb[:, jsl]),
                        rhs=mm_st(g_sb[:, isl]),
                    )
                # just-in-time side work, hidden in the ACT-bound pipe
                if ib == 0:
                    for t in range(nb):
                        emit_vt(b0 + t)
                if b0 == 0 and ib + 1 < N_IB:
                    emit_g(ib + 1)
                if pending is not None:
                    emit_exp_pv(*pending)
                pending = (ib, b0, st_ps)
            emit_exp_pv(*pending)

            # softmax denominators exit from partition 64 via DMA
            nc.sync.dma_start(out=s_d[:, :],
                              in_=s65_sb[DIM_HEAD:DIM_HEAD + 1, :])

    return nc


# Instruction types whose semaphore updates fire in engine program order
# (compute engines are strict-FIFO; DMA completions are async and excluded).
_INORDER_TYPES = (
    "InstMatmult", "InstLdweights", "InstActivation", "InstTensorCopy",
    "InstTensorTensor", "InstTensorScalarPtr", "InstTensorReduce",
    "InstMemset", "InstReciprocal", "InstPartitionBroadcast", "InstIota",
    "InstBnStats", "InstBnAggr", "InstTensorTensorScan", "InstSelect",
    "InstCustomDveAnt",
)


def _strip_redundant_self_waits(nc):
    """Walrus encodes at most ONE sync-wait per compute instruction. Tile
    emits same-engine WAW/WAR waits that are always satisfied by the
    engine's in-order execution; strip exactly those (and merge same-sem
    duplicates) so every instruction carries <=1 wait."""
    from collections import defaultdict

    f = nc.m.functions[0]
    sem_inc_only = {}
    for blk in f.blocks:
        for inst in blk.instructions:
            si = inst.sync_info
            if si is None:
                continue
            for u in si.on_update:
                ok = u.update_mode == "sem-inc"
                nm = u.ant_name
                sem_inc_only[nm] = sem_inc_only.get(nm, True) and ok

    cum = defaultdict(int)  # (engine, sem) -> incs from in-order instructions
    leftover = []
    for blk in f.blocks:
        for inst in blk.instructions:
            si = inst.sync_info
            if si is None:
                continue
            eng = inst.engine
            tname = type(inst).__name__
            if len(si.on_wait) > 1:
                merged = {}
                for w in si.on_wait:
                    k = w.ant_name
                    if k not in merged or w.wait_value > merged[k].wait_value:
                        merged[k] = w
                self_updates = {u.ant_name for u in si.on_update}
                keep = []
                for w in merged.values():
                    if (w.wait_mode == "sem-ge-imm"
                            and sem_inc_only.get(w.ant_name, False)
                            and cum[(eng, w.ant_name)] >= w.wait_value):
                        continue
                    if (tname == "InstDMACopy" and w.ant_name in self_updates):
                        # same HWDGE queue: descriptors complete in FIFO
                        # order, so waiting on this queue's earlier
                        # descriptor is redundant
                        continue
                    keep.append(w)
                if len(keep) != len(si.on_wait):
                    si.on_wait = keep
                if len(keep) > 1 and tname in _INORDER_TYPES:
                    leftover.append((inst.name, tname,
                                     [(w.ant_name, w.wait_value) for w in keep]))
            if tname in _INORDER_TYPES:
                for u in si.on_update:
                    if u.update_mode == "sem-inc":
                        cum[(eng, u.ant_name)] += u.update_value

    # Second pass: any instruction still carrying >1 wait gets all but one
    # split onto standalone single-wait EventSemaphore instructions inserted
    # just before it on the same engine queue.
    import concourse.mybir as mybir
    nsplit = 0
    for blk in f.blocks:
        insts = list(blk.instructions)
        out = []
        changed = False
        for inst in insts:
            si = inst.sync_info
            if si is not None and len(si.on_wait) > 1:
                waits = list(si.on_wait)
                for i, w in enumerate(waits[:-1]):
                    ev = mybir.InstEventSemaphore(
                        name=f"{inst.name}_sw{i}",
                        engine=inst.engine,
                        ins=[],
                        outs=[],
                        sync_info=mybir.SyncInfo(on_wait=[w], on_update=[]),
                    )
                    out.append(ev)
                    nsplit += 1
                si.on_wait = [waits[-1]]
                changed = True
            out.append(inst)
        if changed:
            blk.instructions = out
    return leftover, nsplit


def _ones_arr():
    if MM_DTYPE == "mix":
        import ml_dtypes
        return np.ones((1, N_JC), dtype=ml_dtypes.bfloat16)
    return np.ones((1, N_JC), dtype=np.float32)


def _prep_inputs(x, w_qkv, w_out):
    x_flat = np.ascontiguousarray(x.reshape(DIM, N), dtype=np.float32)
    in_maps = []
    for h in range(HEADS):
        wq = w_qkv[h * DIM_HEAD:(h + 1) * DIM_HEAD].astype(np.float64)
        wk = w_qkv[HIDDEN + h * DIM_HEAD:HIDDEN + (h + 1) * DIM_HEAD].astype(
            np.float64)
        wv = w_qkv[2 * HIDDEN + h * DIM_HEAD:2 * HIDDEN + (h + 1) * DIM_HEAD]
        bmat = (SCALE * (wq.T @ wk)).astype(np.float32)
        wv_t = np.ascontiguousarray(wv.T, dtype=np.float32)
        wo_t = np.ascontiguousarray(
            w_out[:, h * DIM_HEAD:(h + 1) * DIM_HEAD].T, dtype=np.float32)
        in_maps.append({
            "x": x_flat,
            "ones": _ones_arr(),
            "bmat": np.ascontiguousarray(bmat),
            "wv_t": wv_t,
            "wo_t": wo_t,
        })
    return in_maps


def kernel(x, w_qkv, w_out, b_out):
    from concourse.bass_utils import run_bass_kernel_spmd

    if "nc" not in _CACHE:
        nc = _build_nc()
        _strip_redundant_self_waits(nc)
        _CACHE["nc"] = nc
    nc = _CACHE["nc"]

    in_maps = _prep_inputs(np.asarray(x), np.asarray(w_qkv), np.asarray(w_out))
    res = run_bass_kernel_spmd(nc, in_maps, core_ids=list(range(HEADS)))
    total = np.zeros((DIM, N), dtype=np.float64)
    for r in res.results:
        total += r["out"].astype(np.float64) / r["denom"].astype(np.float64)
    total += np.asarray(b_out, dtype=np.float64)[:, None]
    return total.astype(np.float32).reshape(1, DIM, 16, 16, 16)
